# revision 22
# baseline (speedup 1.0000x reference)
"""AttentionBlock kernel for Trainium2 (8 NeuronCores, batch-sharded).

Per sample b:
    q = Wq @ x + bq            [32, N]
    k = Wk @ x + bk            [32, N]
    v = Wv @ x + bv            [256, N]
    attn = softmax(q^T k)      [N, N] (softmax over keys)
    out = gamma * (v @ attn^T) + x

v3: fp8 DoubleRow PV.  S^T [keys, queries] is produced directly
(row-packed 4x via tile_position, K=32), exp needs no max-subtraction
(logits within +-30).  The softmax denominator comes from ones-matmuls
col-packed 4x, broadcast to all 128 partitions via a second ones
matmul whose constant folds in a x32 probability scale.  Probabilities
are then normalized BEFORE the PV matmul (pn = p * 32/den, in [0,32])
so they are fp8_e4m3-safe, and PV runs in DoubleRow perf mode (two
128-key chunks contracted per instruction) for 2x tensor throughput.
The v projection also runs fp8 DoubleRow (full 256-deep contraction per
instruction).  |gamma|/32 is applied in the single fused output op
out = num * g + x (sign(gamma) is folded into Wv/bv on the host).
Software pipeline: PE does [S+den](n) -> PV(n-1) -> den_b(n) so the
DVE pn pass and the Act exp always overlap PE work.
"""

from contextlib import ExitStack

import numpy as np

import concourse.bass as bass
import concourse.mybir as mybir
import concourse.tile as tile
from concourse import bacc
from concourse.bass_utils import run_bass_kernel_spmd

B, C, H, W = 8, 256, 64, 64
N = H * W        # 4096
D = 32           # C // 8
NCORES = 8
P = 128
F32 = mybir.dt.float32
F32R = mybir.dt.float32r
BF16 = mybir.dt.bfloat16
FP8 = mybir.dt.float8e4

NW = 8           # n-chunks of 512 queries
NCH = N // NW    # 512
MP = N // P      # 32 key-chunks of 128
QUAD = 4         # key-chunks per group (row/col packed)
NG = MP // QUAD  # 8 groups

PSCALE = 32.0    # pn = p * PSCALE / den; folded into ones32 + gam


def build_bass():
    nc = bacc.Bacc("TRN2", target_bir_lowering=False, debug=False,
                   enable_asserts=False, num_devices=NCORES)

    x_d = nc.dram_tensor("x", [C, N], F32R, kind="ExternalInput").ap()
    wqT_d = nc.dram_tensor("wqT", [C, D], F32R, kind="ExternalInput").ap()
    wkT_d = nc.dram_tensor("wkT", [C, D], F32R, kind="ExternalInput").ap()
    wvT_d = nc.dram_tensor("wvT", [C, C], F32, kind="ExternalInput").ap()
    bq_d = nc.dram_tensor("bq", [D, 1], F32, kind="ExternalInput").ap()
    bk_d = nc.dram_tensor("bk", [D, 1], F32, kind="ExternalInput").ap()
    bvb_d = nc.dram_tensor("bvb", [P, C], F32, kind="ExternalInput").ap()
    gam_d = nc.dram_tensor("gam", [P, 1], F32, kind="ExternalInput").ap()
    ones16_d = nc.dram_tensor("ones16", [P, D], BF16, kind="ExternalInput").ap()
    ones32_d = nc.dram_tensor("ones32", [P, P], BF16, kind="ExternalInput").ap()
    out_d = nc.dram_tensor("out", [C, N], F32, kind="ExternalOutput").ap()

    with tile.TileContext(nc) as tc, ExitStack() as ctx:
        const = ctx.enter_context(tc.tile_pool(name="const", bufs=1))
        xp = ctx.enter_context(tc.tile_pool(name="xp", bufs=1))
        qk = ctx.enter_context(tc.tile_pool(name="qk", bufs=1))
        vt = ctx.enter_context(tc.tile_pool(name="vt", bufs=1))
        pt = ctx.enter_context(tc.tile_pool(name="pt", bufs=14))
        pnp = ctx.enter_context(tc.tile_pool(name="pnp", bufs=14))
        dsp = ctx.enter_context(tc.tile_pool(name="dsp", bufs=2))
        rdp = ctx.enter_context(tc.tile_pool(name="rdp", bufs=2))
        osp = ctx.enter_context(tc.tile_pool(name="osp", bufs=2))
        ps_st = ctx.enter_context(tc.tile_pool(name="ps_st", bufs=2, space="PSUM"))
        ps_out = ctx.enter_context(tc.tile_pool(name="ps_out", bufs=1, space="PSUM"))
        ps_den = ctx.enter_context(tc.tile_pool(name="ps_den", bufs=2, space="PSUM"))

        # ---- load inputs: small weights first, then x chunks in the
        # order the prologue consumes them ----
        wqT_sb = const.tile([P, 2, D], F32R)
        nc.sync.dma_start(out=wqT_sb[:, 0, :], in_=wqT_d[0:P, :])
        nc.sync.dma_start(out=wqT_sb[:, 1, :], in_=wqT_d[P:C, :])
        wkT_sb = const.tile([P, 2, D], F32R)
        nc.sync.dma_start(out=wkT_sb[:, 0, :], in_=wkT_d[0:P, :])
        nc.sync.dma_start(out=wkT_sb[:, 1, :], in_=wkT_d[P:C, :])
        wvT_sb = const.tile([P, 2, C], F32)
        nc.sync.dma_start(out=wvT_sb[:, 0, :], in_=wvT_d[0:P, :])
        nc.sync.dma_start(out=wvT_sb[:, 1, :], in_=wvT_d[P:C, :])
        bq_sb = const.tile([D, 1], F32)
        nc.sync.dma_start(out=bq_sb, in_=bq_d)
        bk_sb = const.tile([D, 1], F32)
        nc.sync.dma_start(out=bk_sb, in_=bk_d)
        bvb2_sb = const.tile([P, 2, C], F32)
        nc.sync.dma_start(out=bvb2_sb[:, 0, :], in_=bvb_d)
        nc.sync.dma_start(out=bvb2_sb[:, 1, :], in_=bvb_d)
        gam_sb = const.tile([P, 1], F32)
        nc.sync.dma_start(out=gam_sb, in_=gam_d)
        ones16_sb = const.tile([P, D], BF16)
        nc.sync.dma_start(out=ones16_sb, in_=ones16_d)
        ones32_sb = const.tile([P, P], BF16)      # value 1/(32*PSCALE)
        nc.sync.dma_start(out=ones32_sb, in_=ones32_d)

        x_sb = xp.tile([P, 2, N], F32R)           # [128, c-half, 4096]
        for j in range(NW):
            sl = slice(j * NCH, (j + 1) * NCH)
            for ci in range(2):
                nc.sync.dma_start(out=x_sb[:, ci, sl],
                                  in_=x_d[ci * P:(ci + 1) * P, sl])

        # ---- prologue ----
        # q replicated to 4 partition groups; k packed [group j][g, 128]
        q_pack = qk.tile([P, N], BF16)
        k_sb = qk.tile([D, N], BF16)
        k_pack = qk.tile([P, NG, P], BF16)
        x8_sb = xp.tile([P, 2, N], FP8)           # fp8 copy of x for V proj
        wvT8_sb = const.tile([P, 2, C], FP8)
        vT8_sb = vt.tile([P, MP, C], FP8)         # [128, m-chunk, 256]

        _pro = [(ps_st, "stq"), (ps_out, "outq"), (ps_den, "den")]

        def pro_ps(idx, shape, tag_pair):
            pool, tg = _pro[idx % 3]
            return pool.tile(shape, F32, name=f"pro_{tag_pair}_{idx}", tag=tg)

        for j in range(NW):
            sl = slice(j * NCH, (j + 1) * NCH)
            ps_q = pro_ps(j, [D, NCH], "q")
            for ci in range(2):
                nc.tensor.matmul(ps_q, lhsT=wqT_sb[:, ci, :],
                                 rhs=x_sb[:, ci, sl],
                                 start=(ci == 0), stop=(ci == 1))
            nc.vector.tensor_scalar_add(out=q_pack[0:D, sl], in0=ps_q,
                                        scalar1=bq_sb)
            ps_k = pro_ps(j + 1, [D, NCH], "k")
            for ci in range(2):
                nc.tensor.matmul(ps_k, lhsT=wkT_sb[:, ci, :],
                                 rhs=x_sb[:, ci, sl],
                                 start=(ci == 0), stop=(ci == 1))
            nc.vector.tensor_scalar_add(out=k_sb[:, sl], in0=ps_k,
                                        scalar1=bk_sb)

        # replicate q to partition groups 1..3; scatter k into k_pack
        for j in range(1, 4):
            nc.sync.dma_start(out=q_pack[D * j:D * (j + 1), :],
                              in_=q_pack[0:D, :])
        k_view = k_sb.rearrange("p (g j c) -> p g j c", g=NG, j=QUAD, c=P)
        for j in range(4):
            nc.sync.dma_start(out=k_pack[D * j:D * (j + 1), :, :],
                              in_=k_view[:, :, j, :])

        # fp8 copies for the V projection (GpSimd: DVE is the scarcer engine)
        nc.gpsimd.tensor_copy(out=wvT8_sb, in_=wvT_sb)
        nc.gpsimd.tensor_copy(out=x8_sb, in_=x_sb.bitcast(F32))

        # V projection: one fp8 DoubleRow matmul per 128-col chunk
        # (contracts both 128-channel halves in a single instruction)
        for mp2 in range(MP // 2):
            ps_v = pro_ps(mp2, [P, 2, C], "v")
            for mi in range(2):
                m = mp2 * 2 + mi
                msl = slice(m * P, (m + 1) * P)
                nc.tensor.matmul(ps_v[:, mi, :], lhsT=x8_sb[:, :, msl],
                                 rhs=wvT8_sb,
                                 start=True, stop=True,
                                 perf_mode=mybir.MatmulPerfMode.DoubleRow)
            nc.vector.tensor_add(out=vT8_sb[:, mp2 * 2:(mp2 + 1) * 2, :],
                                 in0=ps_v, in1=bvb2_sb)

        # ---- main attention loop ----
        # Software-pipelined one chunk deep: PE order per iteration is
        # [S(n)+den(n,0..6)] -> PV(n-1) -> den(n,7) -> den_b(n); the DVE
        # pn(n) pass lands in iteration n+1's S window.
        pend = {}        # g -> (p_a, p_b) exp'd tiles for current chunk
        pn_done = None   # list of (pn_a, pn_b) per g for chunk n-1
        out_prev = None  # (out_ps, nsl) for chunk n-1
        for n in range(NW + 1):
            if n < NW:
                nsl = slice(n * NCH, (n + 1) * NCH)
                den_ps = ps_den.tile([P, NCH], F32, tag="den",
                                     name=f"den_ps_{n}")
                for g in range(NG):
                    st_a = ps_st.tile([P, 2, NCH], F32, tag="stq",
                                      name=f"st_a_{n}_{g}")
                    st_b = ps_st.tile([P, 2, NCH], F32, tag="stq",
                                      name=f"st_b_{n}_{g}")
                    for j in range(QUAD):
                        dst = st_a if j < 2 else st_b
                        nc.tensor.matmul(dst[:, j % 2, :],
                                         lhsT=k_pack[D * j:D * (j + 1), g, :],
                                         rhs=q_pack[D * j:D * (j + 1), nsl],
                                         start=True, stop=True,
                                         tile_position=(D * j, 0))
                    p_a = pt.tile([P, 2, NCH], BF16, name=f"p_a_{n}_{g}",
                                  tag="p_a")
                    nc.scalar.activation(out=p_a, in_=st_a,
                                         func=mybir.ActivationFunctionType.Exp)
                    p_b = pt.tile([P, 2, NCH], BF16, name=f"p_b_{n}_{g}",
                                  tag="p_b")
                    nc.scalar.activation(out=p_b, in_=st_b,
                                         func=mybir.ActivationFunctionType.Exp)
                    pend[g] = (p_a, p_b)
                    if g > 0:
                        gp_a, gp_b = pend[g - 1]
                        for j in range(QUAD):
                            prhs = (gp_a if j < 2 else gp_b)[:, j % 2, :]
                            nc.tensor.matmul(den_ps[D * j:D * (j + 1), :],
                                             lhsT=ones16_sb, rhs=prhs,
                                             start=(g - 1 == 0), stop=False,
                                             tile_position=(0, D * j))

            if n > 0:
                # PV for chunk n-1 (fp8 DoubleRow, 2 key-chunks/instr)
                out_ps = ps_out.tile([P, 2, NCH], F32, tag="outq",
                                     name=f"out_ps_{n - 1}")
                for g in range(NG):
                    for h in range(2):
                        pn_h = pn_done[g][h]
                        m0 = g * QUAD + 2 * h
                        first = (g == 0 and h == 0)
                        last = (g == NG - 1 and h == 1)
                        for ch in range(2):
                            nc.tensor.matmul(
                                out_ps[:, ch, :],
                                lhsT=vT8_sb[:, m0:m0 + 2,
                                            ch * P:(ch + 1) * P],
                                rhs=pn_h,
                                start=first, stop=last,
                                perf_mode=mybir.MatmulPerfMode.DoubleRow)
                out_prev = (out_prev[0], out_ps, out_prev[2])

            if n < NW:
                # last den group + denominator broadcast + reciprocal
                gp_a, gp_b = pend[NG - 1]
                for j in range(QUAD):
                    prhs = (gp_a if j < 2 else gp_b)[:, j % 2, :]
                    nc.tensor.matmul(den_ps[D * j:D * (j + 1), :],
                                     lhsT=ones16_sb, rhs=prhs,
                                     start=False, stop=True,
                                     tile_position=(0, D * j))
                den_sb = dsp.tile([P, NCH], BF16, name=f"den_sb_{n}",
                                  tag="dsb")
                nc.vector.tensor_copy(out=den_sb, in_=den_ps)
                den_b = ps_den.tile([P, NCH], F32, tag="den",
                                    name=f"den_b_{n}")
                nc.tensor.matmul(den_b, lhsT=ones32_sb, rhs=den_sb,
                                 start=True, stop=True)
                rd_sb = rdp.tile([P, 1, NCH], F32, name=f"rd_{n}", tag="rd")
                nc.vector.reciprocal_approx_fast(out=rd_sb[:, 0, :],
                                                 in_=den_b)

            if n > 0:
                # fused output: out = num * (|gamma|/PSCALE) + x, then DMA
                # (GpSimd so the DVE can keep feeding pn tiles)
                p_nsl, out_ps, _ = out_prev
                out_sb = osp.tile([P, 2, NCH], F32, name=f"out_sb_{n - 1}",
                                  tag="osb")
                for hh in range(2):
                    nc.vector.scalar_tensor_tensor(
                        out=out_sb[:, hh, :], in0=out_ps[:, hh, :],
                        scalar=gam_sb,
                        in1=x_sb[:, hh, p_nsl].bitcast(F32),
                        op0=mybir.AluOpType.mult, op1=mybir.AluOpType.add)
                    nc.sync.dma_start(out=out_d[hh * P:(hh + 1) * P, p_nsl],
                                      in_=out_sb[:, hh, :])

            if n < NW:
                # pn = p * (PSCALE/den)  -> fp8 e4m3 in [0, PSCALE]
                # split across GpSimd (first groups) and DVE (rest)
                rd_bc = rd_sb.broadcast_to((P, 2, NCH))
                pn_cur = []
                for g in range(NG):
                    eng = nc.gpsimd if g < 3 else nc.vector
                    p_a, p_b = pend[g]
                    pn_a = pnp.tile([P, 2, NCH], FP8, name=f"pn_a_{n}_{g}",
                                    tag="pn_a")
                    eng.tensor_mul(out=pn_a, in0=p_a, in1=rd_bc)
                    pn_b = pnp.tile([P, 2, NCH], FP8, name=f"pn_b_{n}_{g}",
                                    tag="pn_b")
                    eng.tensor_mul(out=pn_b, in0=p_b, in1=rd_bc)
                    pn_cur.append((pn_a, pn_b))
                pn_done = pn_cur

            if n < NW:
                out_prev = (nsl, None, None)
    nc.compile()
    return nc


_NC_CACHE = None


def _get_nc():
    global _NC_CACHE
    if _NC_CACHE is None:
        _NC_CACHE = build_bass()
    return _NC_CACHE


def _in_maps(inputs):
    import ml_dtypes
    x = np.ascontiguousarray(np.asarray(inputs["x"], dtype=np.float32))
    wqT = np.ascontiguousarray(np.asarray(inputs["Wq"], np.float32).T)
    wkT = np.ascontiguousarray(np.asarray(inputs["Wk"], np.float32).T)
    wvT = np.ascontiguousarray(np.asarray(inputs["Wv"], np.float32).T)
    bq = np.asarray(inputs["bq"], np.float32).reshape(D, 1).copy()
    bk = np.asarray(inputs["bk"], np.float32).reshape(D, 1).copy()
    gamma = float(np.asarray(inputs["gamma"], np.float32).reshape(()))
    sg = 1.0 if gamma >= 0 else -1.0
    wvT = np.ascontiguousarray(wvT * sg)
    bvb = np.ascontiguousarray(
        sg * np.broadcast_to(np.asarray(inputs["bv"], np.float32)[None, :],
                             (P, C)))
    gam = np.full((P, 1), abs(gamma) / PSCALE, np.float32)
    ones16 = np.ones((P, D), np.float32).astype(ml_dtypes.bfloat16)
    ones32 = np.full((P, P), 1.0 / (32.0 * PSCALE),
                     np.float32).astype(ml_dtypes.bfloat16)
    maps = []
    for b in range(NCORES):
        maps.append({
            "x": np.ascontiguousarray(x[b].reshape(C, N)),
            "wqT": wqT, "wkT": wkT, "wvT": wvT,
            "bq": bq, "bk": bk, "bvb": bvb, "gam": gam,
            "ones16": ones16, "ones32": ones32,
        })
    return maps


def _run(inputs, **kw):
    nc = _get_nc()
    res = run_bass_kernel_spmd(nc, _in_maps(inputs), core_ids=list(range(NCORES)),
                               **kw)
    outs = [res.results[b]["out"].reshape(C, H, W) for b in range(NCORES)]
    return np.stack(outs, axis=0).astype(np.float32), res


def kernel(**inputs) -> np.ndarray:
    out, _ = _run(inputs)
    return out


# revision 26
# speedup vs baseline: 1.6151x; 1.6151x over previous
"""AttentionBlock kernel for Trainium2 (8 NeuronCores, batch-sharded).

Per sample b:
    q = Wq @ x + bq            [32, N]
    k = Wk @ x + bk            [32, N]
    v = Wv @ x + bv            [256, N]
    attn = softmax(q^T k)      [N, N] (softmax over keys)
    out = gamma * (v @ attn^T) + x

Transpose-free layout: S^T [keys, queries] is produced directly, the
softmax denominator (a partition-dim sum) comes from ones-matmuls
col-packed 4x via tile_position, and normalization is deferred to the
[256, N] output (N*C elements instead of N^2).  The K=32 logit matmuls
are row-packed 4x via tile_position, with q replicated to all four
32-partition groups and k scattered into quad layout.  The q/k/v
projections run in float32r (1 cycle/row on the PE for free dims >=
256); q/k/exp-output/PV/denominator run in bf16.  exp needs no
max-subtraction: logits stay within +-30 for unit-scale inputs, safely
inside fp32/bf16 exp range.  1/|gamma| is folded into the denominator
and sign(gamma) into Wv/bv on the host, so normalization is a single
reciprocal + multiply-add per output tile.
"""

from contextlib import ExitStack

import numpy as np

import concourse.bass as bass
import concourse.mybir as mybir
import concourse.tile as tile
from concourse import bacc
from concourse.bass_utils import run_bass_kernel_spmd

B, C, H, W = 8, 256, 64, 64
N = H * W        # 4096
D = 32           # C // 8
NCORES = 8
P = 128
F32 = mybir.dt.float32
F32R = mybir.dt.float32r
BF16 = mybir.dt.bfloat16

NW = 8           # n-chunks of 512 queries
NCH = N // NW    # 512
MP = N // P      # 32 key-chunks of 128
QUAD = 4         # key-chunks per group (row/col packed)
NG = MP // QUAD  # 8 groups


def build_bass():
    nc = bacc.Bacc("TRN2", target_bir_lowering=False, debug=False,
                   enable_asserts=False, num_devices=NCORES)

    x_d = nc.dram_tensor("x", [C, N], F32R, kind="ExternalInput").ap()
    wqT_d = nc.dram_tensor("wqT", [C, D], F32R, kind="ExternalInput").ap()
    wkT_d = nc.dram_tensor("wkT", [C, D], F32R, kind="ExternalInput").ap()
    wvT_d = nc.dram_tensor("wvT", [C, C], F32R, kind="ExternalInput").ap()
    bq_d = nc.dram_tensor("bq", [D, 1], F32, kind="ExternalInput").ap()
    bk_d = nc.dram_tensor("bk", [D, 1], F32, kind="ExternalInput").ap()
    bvb_d = nc.dram_tensor("bvb", [P, C], F32, kind="ExternalInput").ap()
    igam_d = nc.dram_tensor("igam", [P, 1], F32, kind="ExternalInput").ap()
    ones16_d = nc.dram_tensor("ones16", [P, D], BF16, kind="ExternalInput").ap()
    ones32_d = nc.dram_tensor("ones32", [P, P], F32R, kind="ExternalInput").ap()
    out_d = nc.dram_tensor("out", [C, N], F32, kind="ExternalOutput").ap()

    with tile.TileContext(nc) as tc, ExitStack() as ctx:
        const = ctx.enter_context(tc.tile_pool(name="const", bufs=1))
        xp = ctx.enter_context(tc.tile_pool(name="xp", bufs=1))
        qk = ctx.enter_context(tc.tile_pool(name="qk", bufs=1))
        vt = ctx.enter_context(tc.tile_pool(name="vt", bufs=1))
        pt = ctx.enter_context(tc.tile_pool(name="pt", bufs=7))
        op = ctx.enter_context(tc.tile_pool(name="op", bufs=2))
        ps_st = ctx.enter_context(tc.tile_pool(name="ps_st", bufs=2, space="PSUM"))
        ps_out = ctx.enter_context(tc.tile_pool(name="ps_out", bufs=1, space="PSUM"))
        ps_den = ctx.enter_context(tc.tile_pool(name="ps_den", bufs=2, space="PSUM"))

        # ---- load inputs: small weights first, then x chunks in the
        # order the prologue consumes them ----
        wqT_sb = const.tile([P, 2, D], F32R)
        nc.sync.dma_start(out=wqT_sb[:, 0, :], in_=wqT_d[0:P, :])
        nc.sync.dma_start(out=wqT_sb[:, 1, :], in_=wqT_d[P:C, :])
        wkT_sb = const.tile([P, 2, D], F32R)
        nc.sync.dma_start(out=wkT_sb[:, 0, :], in_=wkT_d[0:P, :])
        nc.sync.dma_start(out=wkT_sb[:, 1, :], in_=wkT_d[P:C, :])
        wvT_sb = const.tile([P, 2, C], F32R)
        nc.sync.dma_start(out=wvT_sb[:, 0, :], in_=wvT_d[0:P, :])
        nc.sync.dma_start(out=wvT_sb[:, 1, :], in_=wvT_d[P:C, :])
        bq_sb = const.tile([D, 1], F32)
        nc.sync.dma_start(out=bq_sb, in_=bq_d)
        bk_sb = const.tile([D, 1], F32)
        nc.sync.dma_start(out=bk_sb, in_=bk_d)
        bvb2_sb = const.tile([P, 2, C], F32)
        nc.sync.dma_start(out=bvb2_sb[:, 0, :], in_=bvb_d)
        nc.sync.dma_start(out=bvb2_sb[:, 1, :], in_=bvb_d)
        igam_sb = const.tile([P, 1], F32)
        nc.sync.dma_start(out=igam_sb, in_=igam_d)
        ones16_sb = const.tile([P, D], BF16)
        nc.sync.dma_start(out=ones16_sb, in_=ones16_d)
        ones32_sb = const.tile([P, P], F32R)      # value 1/32
        nc.sync.dma_start(out=ones32_sb, in_=ones32_d)

        x_sb = xp.tile([P, 2, N], F32R)           # [128, c-half, 4096]
        for j in range(NW):
            sl = slice(j * NCH, (j + 1) * NCH)
            for ci in range(2):
                nc.sync.dma_start(out=x_sb[:, ci, sl],
                                  in_=x_d[ci * P:(ci + 1) * P, sl])

        # ---- prologue ----
        # q replicated to 4 partition groups; k packed [group j][g, 128]
        q_pack = qk.tile([P, N], BF16)
        k_sb = qk.tile([D, N], BF16)
        k_pack = qk.tile([P, NG, P], BF16)
        vT16_sb = vt.tile([P, MP, C], BF16)       # [128, m-chunk, 256]

        _pro = [(ps_st, "stq"), (ps_out, "outq"), (ps_den, "den")]

        def pro_ps(idx, shape, tag_pair):
            pool, tg = _pro[idx % 3]
            return pool.tile(shape, F32, name=f"pro_{tag_pair}_{idx}", tag=tg)

        # k first: every S quad needs the full k_pack, so its projection
        # chain (PE matmul -> DVE bias -> scatter DMA) gates the main loop
        for j in range(NW):
            sl = slice(j * NCH, (j + 1) * NCH)
            ps_k = pro_ps(j, [D, NCH], "k")
            for ci in range(2):
                nc.tensor.matmul(ps_k, lhsT=wkT_sb[:, ci, :],
                                 rhs=x_sb[:, ci, sl],
                                 start=(ci == 0), stop=(ci == 1))
            nc.vector.tensor_scalar_add(out=k_sb[:, sl], in0=ps_k,
                                        scalar1=bk_sb)
        k_view = k_sb.rearrange("p (g j c) -> p g j c", g=NG, j=QUAD, c=P)
        for j in range(4):
            nc.sync.dma_start(out=k_pack[D * j:D * (j + 1), :, :],
                              in_=k_view[:, :, j, :])

        def q_proj(j):
            sl = slice(j * NCH, (j + 1) * NCH)
            ps_q = pro_ps(j, [D, NCH], "q")
            for ci in range(2):
                nc.tensor.matmul(ps_q, lhsT=wqT_sb[:, ci, :],
                                 rhs=x_sb[:, ci, sl],
                                 start=(ci == 0), stop=(ci == 1))
            nc.vector.tensor_scalar_add(out=q_pack[0:D, sl], in0=ps_q,
                                        scalar1=bq_sb)
            # replicate this chunk's q to partition groups 1..3
            for r in range(1, 4):
                nc.sync.dma_start(out=q_pack[D * r:D * (r + 1), sl],
                                  in_=q_pack[0:D, sl])

        def v_proj(mp2):
            ps_v = pro_ps(mp2, [P, 2, C], "v")
            for mi in range(2):
                m = mp2 * 2 + mi
                msl = slice(m * P, (m + 1) * P)
                for ci in range(2):
                    nc.tensor.matmul(ps_v[:, mi, :], lhsT=x_sb[:, ci, msl],
                                     rhs=wvT_sb[:, ci, :],
                                     start=(ci == 0), stop=(ci == 1))
            nc.vector.tensor_add(out=vT16_sb[:, mp2 * 2:(mp2 + 1) * 2, :],
                                 in0=ps_v, in1=bvb2_sb)

        # chunk 0's q, then early v chunks (PV(0) consumes vT16 from m=0
        # upward), then the later q chunks interleaved with the rest of v
        q_proj(0)
        for mp2 in range(8):
            v_proj(mp2)
        q_proj(1)
        q_proj(2)
        for mp2 in range(8, MP // 2):
            v_proj(mp2)
        for j in range(3, NW):
            q_proj(j)

        # ---- main attention loop ----
        # Software-pipelined per chunk: quad g's S^T+exp issue before
        # quad g-1's PV/den so the PE never waits on the ScalarE exp.
        for n in range(NW):
            nsl = slice(n * NCH, (n + 1) * NCH)
            out_ps = ps_out.tile([P, 2, NCH], F32, tag="outq")   # 2 banks
            den_ps = ps_den.tile([P, NCH], F32, tag="den")       # 1 bank
            pend = {}
            for g in range(NG + 2):
                if g < NG:
                    st_a = ps_st.tile([P, 2, NCH], F32, tag="stq")
                    st_b = ps_st.tile([P, 2, NCH], F32, tag="stq")
                    for j in range(QUAD):
                        dst = st_a if j < 2 else st_b
                        nc.tensor.matmul(dst[:, j % 2, :],
                                         lhsT=k_pack[D * j:D * (j + 1), g, :],
                                         rhs=q_pack[D * j:D * (j + 1), nsl],
                                         start=True, stop=True,
                                         tile_position=(D * j, 0))
                    p_a = pt.tile([P, 2, NCH], BF16)
                    nc.scalar.activation(out=p_a, in_=st_a,
                                         func=mybir.ActivationFunctionType.Exp)
                    p_b = pt.tile([P, 2, NCH], BF16)
                    nc.scalar.activation(out=p_b, in_=st_b,
                                         func=mybir.ActivationFunctionType.Exp)
                    pend[g] = (p_a, p_b)
                if g > 1:
                    gg = g - 2
                    p_a, p_b = pend.pop(gg)
                    first = (gg == 0)
                    last = (gg == NG - 1)
                    if last:
                        # last quad: den first so the den->rd chain can
                        # overlap the final PV group
                        for j in range(QUAD):
                            prhs = (p_a if j < 2 else p_b)[:, j % 2, :]
                            nc.tensor.matmul(den_ps[D * j:D * (j + 1), :],
                                             lhsT=ones16_sb, rhs=prhs,
                                             start=first, stop=last,
                                             tile_position=(0, D * j))
                        den_sb = op.tile([P, NCH], F32R, tag="dsb",
                                         name=f"den_sb_{n}")
                        nc.vector.tensor_scalar_mul(out=den_sb, in0=den_ps,
                                                    scalar1=igam_sb)
                    for j in range(QUAD):
                        m = gg * QUAD + j
                        prhs = (p_a if j < 2 else p_b)[:, j % 2, :]
                        nc.tensor.matmul(out_ps[:, 0, :],
                                         lhsT=vT16_sb[:, m, 0:P], rhs=prhs,
                                         start=(first and j == 0),
                                         stop=(last and j == QUAD - 1))
                        nc.tensor.matmul(out_ps[:, 1, :],
                                         lhsT=vT16_sb[:, m, P:C], rhs=prhs,
                                         start=(first and j == 0),
                                         stop=(last and j == QUAD - 1))
                        if last and j == 1:
                            den_b = ps_den.tile([P, NCH], F32, tag="den",
                                                name=f"den_b_{n}")
                            nc.tensor.matmul(den_b, lhsT=ones32_sb,
                                             rhs=den_sb,
                                             start=True, stop=True)
                    if not last:
                        for j in range(QUAD):
                            prhs = (p_a if j < 2 else p_b)[:, j % 2, :]
                            nc.tensor.matmul(den_ps[D * j:D * (j + 1), :],
                                             lhsT=ones16_sb, rhs=prhs,
                                             start=first, stop=last,
                                             tile_position=(0, D * j))
            # rd = |gamma|/den (den_b computed inside the last PV group;
            # sign(gamma) is folded into Wv/bv host-side)
            rd_sb = op.tile([P, NCH], F32)
            nc.vector.reciprocal_approx_fast(out=rd_sb, in_=den_b)
            # normalize: out = rd * num + x
            out_sb = op.tile([P, 2, NCH], F32)
            for hh in range(2):
                nc.vector.tensor_mul(out=out_sb[:, hh, :],
                                     in0=out_ps[:, hh, :], in1=rd_sb)
            for hh in range(2):
                nc.vector.tensor_add(out=out_sb[:, hh, :],
                                     in0=out_sb[:, hh, :],
                                     in1=x_sb[:, hh, nsl].bitcast(F32))
                nc.sync.dma_start(out=out_d[hh * P:(hh + 1) * P, nsl],
                                  in_=out_sb[:, hh, :])
    nc.compile()
    return nc


_NC_CACHE = None


def _get_nc():
    global _NC_CACHE
    if _NC_CACHE is None:
        _NC_CACHE = build_bass()
    return _NC_CACHE


def _in_maps(inputs):
    import ml_dtypes
    x = np.ascontiguousarray(np.asarray(inputs["x"], dtype=np.float32))
    wqT = np.ascontiguousarray(np.asarray(inputs["Wq"], np.float32).T)
    wkT = np.ascontiguousarray(np.asarray(inputs["Wk"], np.float32).T)
    wvT = np.ascontiguousarray(np.asarray(inputs["Wv"], np.float32).T)
    bq = np.asarray(inputs["bq"], np.float32).reshape(D, 1).copy()
    bk = np.asarray(inputs["bk"], np.float32).reshape(D, 1).copy()
    gamma = float(np.asarray(inputs["gamma"], np.float32).reshape(()))
    sg = 1.0 if gamma >= 0 else -1.0
    wvT = np.ascontiguousarray(wvT * sg)
    bvb = np.ascontiguousarray(
        sg * np.broadcast_to(np.asarray(inputs["bv"], np.float32)[None, :],
                             (P, C)))
    igam = np.full((P, 1), 1.0 / max(abs(gamma), 1e-12), np.float32)
    ones16 = np.ones((P, D), np.float32).astype(ml_dtypes.bfloat16)
    ones32 = np.full((P, P), 1.0 / 32.0, np.float32)
    maps = []
    for b in range(NCORES):
        maps.append({
            "x": np.ascontiguousarray(x[b].reshape(C, N)),
            "wqT": wqT, "wkT": wkT, "wvT": wvT,
            "bq": bq, "bk": bk, "bvb": bvb, "igam": igam,
            "ones16": ones16, "ones32": ones32,
        })
    return maps


def _run(inputs, **kw):
    nc = _get_nc()
    res = run_bass_kernel_spmd(nc, _in_maps(inputs), core_ids=list(range(NCORES)),
                               **kw)
    outs = [res.results[b]["out"].reshape(C, H, W) for b in range(NCORES)]
    return np.stack(outs, axis=0).astype(np.float32), res


def kernel(**inputs) -> np.ndarray:
    out, _ = _run(inputs)
    return out

